# revision 1
# baseline (speedup 1.0000x reference)
"""MLA (multi-head latent attention) Trainium2 Bass kernel.

Sharding: 8 cores = batch(2) x head-groups(4 heads each).
Latent projections (W_dq/W_dkv) replicated within a batch group; heads
tensor-parallel; final projection row-split with host-side partial sum.

All matmuls run in bf16 (full PE rate) with fp32 PSUM accumulation.
Attention is computed in "transposed score" orientation S_T[j, q] so the
softmax denominator folds into the PE via a ones-column appended to V and
no cross-partition reductions are needed. Softmax skips max-subtraction
(scores are O(1) here; exp is computed in fp32 from PSUM).
"""

import sys
import numpy as np
import ml_dtypes

for _p in ("/opt/trn_rl_repo", "/root/.axon_site/_ro/trn_rl_repo"):
    if _p not in sys.path:
        sys.path.append(_p)

BF16 = ml_dtypes.bfloat16

D_MODEL = 2048
SEQ = 2048
BATCH = 2
N_HEADS = 16
D_HEAD = 128
D_KV = 512
D_ROPE = 64
ROPE_BASE = 10000.0
EPS = 1e-5
H_LOC = 4          # heads per core
N_CORES = 8

_BUILD_CACHE = {}


def build_program(reps: int = 1):
    """Build (and cache) the per-core Bass program. SPMD: same program on
    all 8 cores; per-core data differs via the input maps."""
    if reps in _BUILD_CACHE:
        return _BUILD_CACHE[reps]

    import concourse.bass as bass  # noqa: F401
    import concourse.mybir as mybir
    from concourse import bacc
    from concourse.tile import TileContext
    from concourse.masks import make_identity
    from contextlib import ExitStack

    f32 = mybir.dt.float32
    bf16 = mybir.dt.bfloat16
    AF = mybir.ActivationFunctionType
    OP = mybir.AluOpType

    nc = bacc.Bacc(num_devices=8)

    xT = nc.declare_dram_parameter("xT", [D_MODEL, SEQ], bf16, isOutput=False)
    wdq = nc.declare_dram_parameter("wdq", [D_MODEL, D_KV], bf16, isOutput=False)
    wdkv = nc.declare_dram_parameter("wdkv", [D_MODEL, D_KV], bf16, isOutput=False)
    wq = nc.declare_dram_parameter("wq", [D_KV, H_LOC * 128], bf16, isOutput=False)
    wuk2 = nc.declare_dram_parameter("wuk2", [D_KV, 2 * 128], bf16, isOutput=False)
    wkr2 = nc.declare_dram_parameter("wkr2", [D_MODEL, 2 * 128], bf16, isOutput=False)
    wuv = nc.declare_dram_parameter("wuv", [D_KV, H_LOC * 128], bf16, isOutput=False)
    wout = nc.declare_dram_parameter("wout", [H_LOC * 128, D_MODEL], bf16, isOutput=False)
    xq = nc.declare_dram_parameter("xq", [D_MODEL, 512], bf16, isOutput=False)
    mult = nc.declare_dram_parameter("mult", [128, 2 * SEQ], bf16, isOutput=False)
    masks = nc.declare_dram_parameter("masks", [128, 4 * 512], bf16, isOutput=False)
    y = nc.declare_dram_parameter("y", [SEQ, D_MODEL], bf16, isOutput=True)
    csrc = nc.dram_tensor("csrc", [2 * D_KV, 512], bf16)
    cdst = nc.dram_tensor("cdst", [4, 2 * D_KV, 512], bf16)

    SCALE = 1.0 / float(np.sqrt(np.float32(D_HEAD)))
    NKT = D_MODEL // 128    # 16 k-tiles over d_model
    NLT = D_KV // 128       # 4  k-tiles over latent
    NSN = SEQ // 512        # 4  sequence chunks
    NQB = SEQ // 512        # 4  q blocks
    VROW = D_HEAD + 1       # 129: V row with ones column

    with TileContext(nc) as tc, ExitStack() as top:
        pp = top.enter_context(tc.tile_pool(name="persist", bufs=1))
        # persistent SBUF tensors (live until the end)
        qt_sb = pp.tile([128, H_LOC * SEQ], bf16, tag="qt")
        kt_sb = pp.tile([128, H_LOC * SEQ], bf16, tag="kt")
        v_sb = pp.tile([128, (SEQ // 128) * H_LOC * VROW], bf16, tag="v")

        for _rep in range(reps):
            # ---------------- Phase A: latents + projections ----------------
            with ExitStack() as pa:
                pA = pa.enter_context(tc.tile_pool(name="phA", bufs=1))
                pCp = pa.enter_context(tc.tile_pool(name="pCp", bufs=4))
                pSq = pa.enter_context(tc.tile_pool(name="pSq", bufs=4))
                pCn = pa.enter_context(tc.tile_pool(name="pCn", bufs=5))
                pSt = pa.enter_context(tc.tile_pool(name="pSt", bufs=2))
                psA = pa.enter_context(tc.tile_pool(name="psA", bufs=6, space="PSUM"))
                psS = pa.enter_context(tc.tile_pool(name="psS", bufs=2, space="PSUM"))

                xq_sb = pA.tile([128, NKT * 512], bf16, tag="xq")
                cq_full = pA.tile([128, NLT * SEQ], bf16, tag="cqf")
                ckv_full = pA.tile([128, NLT * SEQ], bf16, tag="ckvf")
                wdq_sb = pA.tile([128, NKT * D_KV], bf16, tag="wdq")
                wdkv_sb = pA.tile([128, NKT * D_KV], bf16, tag="wdkv")
                wq_sb = pA.tile([128, NLT * 512], bf16, tag="wq")
                wuk2_sb = pA.tile([128, NLT * 256], bf16, tag="wuk2")
                wkr2_sb = pA.tile([128, NKT * 256], bf16, tag="wkr2")
                wuv_sb = pA.tile([128, NLT * 512], bf16, tag="wuv")
                mult_sb = pA.tile([128, 2 * SEQ], bf16, tag="mult")
                ones_sb = pA.tile([128, 1], bf16, tag="ones")

                for kt in range(NKT):
                    nc.sync.dma_start(out=xq_sb[:, kt * 512:(kt + 1) * 512],
                                      in_=xq[kt * 128:(kt + 1) * 128, :])
                    nc.gpsimd.dma_start(out=wdq_sb[:, kt * D_KV:(kt + 1) * D_KV],
                                        in_=wdq[kt * 128:(kt + 1) * 128, :])
                    nc.gpsimd.dma_start(out=wdkv_sb[:, kt * D_KV:(kt + 1) * D_KV],
                                        in_=wdkv[kt * 128:(kt + 1) * 128, :])
                    nc.gpsimd.dma_start(out=wkr2_sb[:, kt * 256:(kt + 1) * 256],
                                        in_=wkr2[kt * 128:(kt + 1) * 128, :])
                for lt in range(NLT):
                    nc.sync.dma_start(out=wq_sb[:, lt * 512:(lt + 1) * 512],
                                      in_=wq[lt * 128:(lt + 1) * 128, :])
                    nc.sync.dma_start(out=wuk2_sb[:, lt * 256:(lt + 1) * 256],
                                      in_=wuk2[lt * 128:(lt + 1) * 128, :])
                    nc.sync.dma_start(out=wuv_sb[:, lt * 512:(lt + 1) * 512],
                                      in_=wuv[lt * 128:(lt + 1) * 128, :])
                nc.sync.dma_start(out=mult_sb[:], in_=mult[:, :])
                nc.gpsimd.memset(ones_sb[:], 1.0)
                eps_sb = pA.tile([1, 1], f32, tag="eps")
                nc.gpsimd.memset(eps_sb[:], EPS)
                # ones columns of V (d column 128 of each 129-wide row block)
                v_ones = v_sb.rearrange("p (k d) -> p k d", d=VROW)[:, :, 128:129]
                nc.vector.memset(v_ones, 1.0)

                # --- local latent quarter from xq, rmsnorm, send to gather ---
                for ci, (cname, wd_sb) in enumerate((("q", wdq_sb), ("kv", wdkv_sb))):
                    cps_l, sq_l = [], []
                    for lt in range(NLT):
                        cp = psA.tile([128, 512], f32, tag="mm")
                        for kt in range(NKT):
                            nc.tensor.matmul(
                                cp[:],
                                wd_sb[:, kt * D_KV + lt * 128: kt * D_KV + (lt + 1) * 128],
                                xq_sb[:, kt * 512:(kt + 1) * 512],
                                start=(kt == 0), stop=(kt == NKT - 1))
                        cps = pCp.tile([128, 512], f32, tag="cpre", name=f"cpre{cname}{lt}")
                        nc.vector.tensor_copy(cps[:], cp[:])
                        sq = pSq.tile([128, 512], bf16, tag="sq", name=f"sq{cname}{lt}")
                        nc.vector.tensor_tensor(sq[:], cps[:], cps[:], OP.mult)
                        cps_l.append(cps)
                        sq_l.append(sq)
                    ss = psS.tile([1, 512], f32, tag="stat")
                    for lt in range(NLT):
                        nc.tensor.matmul(ss[:], ones_sb[:], sq_l[lt][:],
                                         start=(lt == 0), stop=(lt == NLT - 1))
                    lnv = pSt.tile([1, 512], f32, tag="ln")
                    nc.scalar.activation(lnv[:], ss[:], AF.Ln, scale=1.0 / D_KV, bias=eps_sb[:])
                    rstd = pSt.tile([1, 512], f32, tag="rstd")
                    nc.scalar.activation(rstd[:], lnv[:], AF.Exp, scale=-0.5)
                    bstd = pSt.tile([128, 512], f32, tag="bstd")
                    nc.gpsimd.partition_broadcast(bstd[:], rstd[:])
                    for lt in range(NLT):
                        cnt = pCp.tile([128, 512], bf16, tag="cnloc", name=f"cnloc{cname}{lt}")
                        nc.vector.tensor_tensor(cnt[:], cps_l[lt][:], bstd[:], OP.mult)
                        nc.sync.dma_start(
                            out=csrc[ci * D_KV + lt * 128: ci * D_KV + (lt + 1) * 128, :],
                            in_=cnt[:])

                # --- kr projections from streamed xT (independent of gather) ---
                for wave in range(2):
                    kps = {}
                    for p in range(2):
                        for sn in (2 * wave, 2 * wave + 1):
                            kps[(p, sn)] = psA.tile([128, 512], f32, tag="mm",
                                                    name=f"kr{wave}{p}{sn}")
                    for kt in range(NKT):
                        xtile = pCp.tile([128, SEQ], bf16, tag="xs", name=f"xs{wave}{kt}")
                        nc.sync.dma_start(out=xtile[:], in_=xT[kt * 128:(kt + 1) * 128, :])
                        for p in range(2):
                            for sn in (2 * wave, 2 * wave + 1):
                                nc.tensor.matmul(
                                    kps[(p, sn)][:],
                                    wkr2_sb[:, kt * 256 + p * 128: kt * 256 + (p + 1) * 128],
                                    xtile[:, sn * 512:(sn + 1) * 512],
                                    start=(kt == 0), stop=(kt == NKT - 1))
                    for p in range(2):
                        h0, h1 = 2 * p, 2 * p + 1
                        for sn in (2 * wave, 2 * wave + 1):
                            kp = kps[(p, sn)]
                            m0 = mult_sb[:, 0 * SEQ + sn * 512: 0 * SEQ + (sn + 1) * 512]
                            m1 = mult_sb[:, 1 * SEQ + sn * 512: 1 * SEQ + (sn + 1) * 512]
                            k0 = kt_sb[:, h0 * SEQ + sn * 512: h0 * SEQ + (sn + 1) * 512]
                            k1 = kt_sb[:, h1 * SEQ + sn * 512: h1 * SEQ + (sn + 1) * 512]
                            nc.vector.tensor_tensor(k0[64:128, :], kp[64:128, :], m0[64:128, :], OP.mult)
                            nc.vector.tensor_tensor(k1[0:64, :], kp[0:64, :], m1[0:64, :], OP.mult)

                # --- all-gather the latent quarters across the 4-core group ---
                nc.gpsimd.collective_compute(
                    "AllGather", OP.bypass,
                    replica_groups=[[0, 1, 2, 3], [4, 5, 6, 7]],
                    ins=[csrc[:, :]], outs=[cdst[:, :, :]])
                for ci, cfull in ((0, cq_full), (1, ckv_full)):
                    for sn in range(NSN):
                        for lt in range(NLT):
                            nc.sync.dma_start(
                                out=cfull[:, lt * SEQ + sn * 512: lt * SEQ + (sn + 1) * 512],
                                in_=cdst[sn, ci * D_KV + lt * 128: ci * D_KV + (lt + 1) * 128, :])

                # --- projections from gathered latents ---
                for sn in range(NSN):
                    def cnq(lt):
                        return cq_full[:, lt * SEQ + sn * 512: lt * SEQ + (sn + 1) * 512]
                    def cnkv(lt):
                        return ckv_full[:, lt * SEQ + sn * 512: lt * SEQ + (sn + 1) * 512]
                    for hl in range(H_LOC):
                        qp = psA.tile([128, 512], f32, tag="mm")
                        for lt in range(NLT):
                            nc.tensor.matmul(
                                qp[:],
                                wq_sb[:, lt * 512 + hl * 128: lt * 512 + (hl + 1) * 128],
                                cnq(lt),
                                start=(lt == 0), stop=(lt == NLT - 1))
                        nc.vector.tensor_tensor(
                            qt_sb[:, hl * SEQ + sn * 512: hl * SEQ + (sn + 1) * 512],
                            qp[:], mult_sb[:, (hl % 2) * SEQ + sn * 512: (hl % 2) * SEQ + (sn + 1) * 512],
                            OP.mult)
                    for p in range(2):
                        h0, h1 = 2 * p, 2 * p + 1
                        up = psA.tile([128, 512], f32, tag="mm")
                        for lt in range(NLT):
                            nc.tensor.matmul(
                                up[:],
                                wuk2_sb[:, lt * 256 + p * 128: lt * 256 + (p + 1) * 128],
                                cnkv(lt),
                                start=(lt == 0), stop=(lt == NLT - 1))
                        m0 = mult_sb[:, 0 * SEQ + sn * 512: 0 * SEQ + (sn + 1) * 512]
                        m1 = mult_sb[:, 1 * SEQ + sn * 512: 1 * SEQ + (sn + 1) * 512]
                        k0 = kt_sb[:, h0 * SEQ + sn * 512: h0 * SEQ + (sn + 1) * 512]
                        k1 = kt_sb[:, h1 * SEQ + sn * 512: h1 * SEQ + (sn + 1) * 512]
                        nc.vector.tensor_tensor(k0[0:64, :], up[0:64, :], m0[0:64, :], OP.mult)
                        nc.vector.tensor_tensor(k1[64:128, :], up[64:128, :], m1[64:128, :], OP.mult)
                    for st in range(4):
                        s_tile = sn * 4 + st
                        vp = psA.tile([128, 512], f32, tag="mm")
                        for lt in range(NLT):
                            nc.tensor.matmul(
                                vp[:],
                                cnkv(lt)[:, st * 128:(st + 1) * 128],
                                wuv_sb[:, lt * 512:(lt + 1) * 512],
                                start=(lt == 0), stop=(lt == NLT - 1))
                        vdst = v_sb.rearrange("p (k d) -> p k d", d=VROW)[
                            :, s_tile * H_LOC:(s_tile + 1) * H_LOC, 0:128]
                        vsrc = vp.rearrange("p (k d) -> p k d", d=128)
                        nc.vector.tensor_copy(vdst, vsrc)

            # ---------------- Phase B: attention + output projection ----------------
            with ExitStack() as pb:
                pB = pb.enter_context(tc.tile_pool(name="phB", bufs=1))
                pOt = pb.enter_context(tc.tile_pool(name="pOt", bufs=2))
                pEs = pb.enter_context(tc.tile_pool(name="pEs", bufs=6))
                pOd = pb.enter_context(tc.tile_pool(name="pOd", bufs=4))
                pYs = pb.enter_context(tc.tile_pool(name="pYs", bufs=3))
                psB = pb.enter_context(tc.tile_pool(name="psB", bufs=4, space="PSUM"))
                psO = pb.enter_context(tc.tile_pool(name="psO", bufs=1, space="PSUM"))

                masks_sb = pB.tile([128, 4 * 512], bf16, tag="masks")
                ident_sb = pB.tile([128, 128], bf16, tag="ident")
                wout_sb = pB.tile([128, NLT * D_MODEL], bf16, tag="wout")
                nc.sync.dma_start(out=masks_sb[:], in_=masks[:, :])
                for f in range(NLT):
                    nc.sync.dma_start(out=wout_sb[:, f * D_MODEL:(f + 1) * D_MODEL],
                                      in_=wout[f * 128:(f + 1) * 128, :])
                make_identity(nc, ident_sb[:])

                for qb in range(NQB):
                    njt = (qb + 1) * 4
                    otc = [pOt.tile([128, 512], bf16, tag=f"otc{f}", name=f"otc{qb}_{f}")
                           for f in range(H_LOC)]
                    for hl in range(H_LOC):
                        ob = psO.tile([128, 2048], f32, tag="obank")
                        for jt in range(njt):
                            kd = jt - qb * 4
                            c0 = max(kd, 0) * 128  # first live q column of this j-tile
                            sp = psB.tile([128, 512], f32, tag="mm", name=f"s{qb}{hl}{jt}")
                            nc.tensor.matmul(
                                sp[:, c0:],
                                kt_sb[:, hl * SEQ + jt * 128: hl * SEQ + (jt + 1) * 128],
                                qt_sb[:, hl * SEQ + qb * 512 + c0: hl * SEQ + (qb + 1) * 512],
                                start=True, stop=True)
                            es = pEs.tile([128, 512], bf16, tag="expS", name=f"e{qb}{hl}{jt}")
                            nc.scalar.activation(es[:, c0:], sp[:, c0:], AF.Exp, scale=SCALE)
                            if kd >= 0:
                                nc.vector.tensor_tensor(
                                    es[:, c0:], es[:, c0:],
                                    masks_sb[:, kd * 512 + c0:(kd + 1) * 512], OP.mult)
                            vsl = v_sb[:, jt * H_LOC * VROW + hl * VROW:
                                       jt * H_LOC * VROW + (hl + 1) * VROW]
                            for qs in range(4):
                                if kd > qs:
                                    continue  # q-sub-block entirely below the causal mask
                                nc.tensor.matmul(
                                    ob[:, qs * 512: qs * 512 + VROW],
                                    es[:, qs * 128:(qs + 1) * 128],
                                    vsl,
                                    start=(jt == 0), stop=(jt == qb * 4 + qs))
                        for qs in range(4):
                            zr = pOd.tile([128, 1], f32, tag="zr")
                            nc.vector.reciprocal(zr[:], ob[:, qs * 512 + 128: qs * 512 + VROW])
                            od = pOd.tile([128, 128], bf16, tag="od")
                            nc.vector.tensor_scalar_mul(od[:], ob[:, qs * 512: qs * 512 + 128], zr[:])
                            tp = psB.tile([128, 128], bf16, tag="mm", name=f"tp{qb}{hl}{qs}")
                            nc.tensor.transpose(tp[:], od[:], ident_sb[:])
                            nc.vector.tensor_copy(otc[hl][:, qs * 128:(qs + 1) * 128], tp[:])

                    for st in range(4):
                        row0 = qb * 512 + st * 128
                        for ncol in range(4):
                            yp = psB.tile([128, 512], f32, tag="mm", name=f"y{qb}{st}{ncol}")
                            for f in range(H_LOC):
                                nc.tensor.matmul(
                                    yp[:],
                                    otc[f][:, st * 128:(st + 1) * 128],
                                    wout_sb[:, f * D_MODEL + ncol * 512: f * D_MODEL + (ncol + 1) * 512],
                                    start=(f == 0), stop=(f == H_LOC - 1))
                            ys = pYs.tile([128, 512], bf16, tag="ysb")
                            if (st + ncol) % 2 == 0:
                                nc.vector.tensor_copy(ys[:], yp[:])
                            else:
                                nc.scalar.copy(ys[:], yp[:])
                            nc.sync.dma_start(out=y[row0:row0 + 128, ncol * 512:(ncol + 1) * 512],
                                              in_=ys[:])

    nc.finalize()
    _BUILD_CACHE[reps] = nc
    return nc


def _rope_mult():
    """r[s, d] = cos + sin rope multiplier, transposed to [64, SEQ]."""
    half = D_ROPE // 2
    theta = 1.0 / (ROPE_BASE ** (np.arange(0, D_HEAD, 2, dtype=np.float32) / D_HEAD))
    idx = np.arange(SEQ, dtype=np.float32)[:, None] * theta[None, :]
    r = np.tile(np.cos(idx[:, :half]), (1, 2)) + np.tile(np.sin(idx[:, :half]), (1, 2))
    return np.ascontiguousarray(r.T).astype(np.float32)  # [64, SEQ]


def make_inputs(x, W_dq, W_uq, W_dkv, W_uk, W_uv, W_qr, W_kr, g_q, g_kv, W_out, b_out):
    """Host-side sharding/packing: per-core input maps."""
    rT = _rope_mult()
    mult = np.empty((128, 2 * SEQ), np.float32)
    mult[0:64, 0:SEQ] = 1.0
    mult[64:128, 0:SEQ] = rT
    mult[0:64, SEQ:] = rT
    mult[64:128, SEQ:] = 1.0
    mult = mult.astype(BF16)

    masks = np.zeros((128, 4 * 512), np.float32)
    jl = np.arange(128)[:, None]
    ql = np.arange(512)[None, :]
    for k in range(4):
        masks[:, k * 512:(k + 1) * 512] = (ql >= 128 * k + jl)
    masks = masks.astype(BF16)

    gq = g_q.astype(np.float32)[:, None]
    gkv = g_kv.astype(np.float32)[:, None]
    Wuq_g = W_uq * gq
    Wqr_g = W_qr * gq
    Wuk_g = W_uk * gkv
    Wuv_g = W_uv * gkv

    in_maps = []
    for core in range(N_CORES):
        b = core // 4
        g = core % 4
        heads = [4 * g + i for i in range(H_LOC)]

        xb = np.ascontiguousarray(x[b].T).astype(BF16)  # [d_model, seq]

        wq_pack = np.empty((D_KV, H_LOC * 128), np.float32)
        for hl, h in enumerate(heads):
            a = Wuq_g[:, h * 64:(h + 1) * 64]
            r = Wqr_g[:, h * 64:(h + 1) * 64]
            blk = np.concatenate([a, r], axis=1) if hl % 2 == 0 else np.concatenate([r, a], axis=1)
            wq_pack[:, hl * 128:(hl + 1) * 128] = blk

        wuk2 = np.empty((D_KV, 256), np.float32)
        wkr2 = np.empty((D_MODEL, 256), np.float32)
        for p in range(2):
            h0, h1 = heads[2 * p], heads[2 * p + 1]
            wuk2[:, p * 128: p * 128 + 64] = Wuk_g[:, h0 * 64:(h0 + 1) * 64]
            wuk2[:, p * 128 + 64: p * 128 + 128] = Wuk_g[:, h1 * 64:(h1 + 1) * 64]
            # rot halves swapped: odd head's rope block first
            wkr2[:, p * 128: p * 128 + 64] = W_kr[:, h1 * 64:(h1 + 1) * 64]
            wkr2[:, p * 128 + 64: p * 128 + 128] = W_kr[:, h0 * 64:(h0 + 1) * 64]

        wuv_pack = np.concatenate(
            [Wuv_g[:, h * 128:(h + 1) * 128] for h in heads], axis=1)
        wout_pack = np.concatenate(
            [W_out[h * 128:(h + 1) * 128, :] for h in heads], axis=0)

        in_maps.append({
            "xT": xb,
            "xq": np.ascontiguousarray(xb[:, g * 512:(g + 1) * 512]),
            "wdq": W_dq.astype(BF16),
            "wdkv": W_dkv.astype(BF16),
            "wq": wq_pack.astype(BF16),
            "wuk2": wuk2.astype(BF16),
            "wkr2": wkr2.astype(BF16),
            "wuv": wuv_pack.astype(BF16),
            "wout": wout_pack.astype(BF16),
            "mult": mult,
            "masks": masks,
        })
    return in_maps


def kernel(**inputs):
    inputs = {k: np.asarray(v) for k, v in inputs.items()}
    in_maps = make_inputs(
        inputs["x"], inputs["W_dq"], inputs["W_uq"], inputs["W_dkv"],
        inputs["W_uk"], inputs["W_uv"], inputs["W_qr"], inputs["W_kr"],
        inputs["g_q"], inputs["g_kv"], inputs["W_out"], inputs["b_out"])

    nc = build_program(reps=1)
    from concourse.bass_utils import run_bass_kernel_spmd
    res = run_bass_kernel_spmd(nc, in_maps, list(range(N_CORES)))

    b_out = inputs["b_out"].astype(np.float32)
    out = np.zeros((BATCH, SEQ, D_MODEL), np.float32)
    for core in range(N_CORES):
        out[core // 4] += res.results[core]["y"].astype(np.float32)
    out += b_out[None, None, :]
    return out

